# revision 12
# baseline (speedup 1.0000x reference)
"""Trainium2 Bass kernel for the GNN message-passing module.

Per-sample pipeline (data-parallel: one batch element per NeuronCore):
  1. pass 1: segment sums via one-hot matmul on PE. The pixel-major
     transposed x (fp8 e4m3) and the pixel-major one-hot (fp8, exact)
     are prepared on the host, so pass 1 is a pure 128-matmul
     accumulation chain -- no on-chip transposes, no PSUM evacuation.
  2. small "middle" stage: means, M=W@W^T, Mahalanobis adjacency folded
     into a (K, C_out) table: table2T = adj-weighted conv'd means,
  3. pass 2: out = conv_w @ x + table2T[index] via PE matmuls (the
     gather is a one-hot matmul accumulated into the same PSUM as the
     1x1 conv). Output is written bf16 and upcast on the host.

Math notes:
  adj[i,j] = exp(-(m_j-m_i)^T M (m_j-m_i)) with zero diagonal, M=W W^T.
  Using G = means @ M @ means^T, g = diag(G):
    adj[i,j] = exp(2G_ij - g_i - g_j) - delta_ij
  agg = adj @ means  =>  out += conv_w @ agg[index]
  table2T[k,:] = e^{-g_k} * (aggT_raw^T @ conv_w^T)[k,:] - (means @ conv_w^T)[k,:]
  where aggT_raw[:,i] = sum_j B[j,i] * (e^{-g_j} means[j,:]),
        B[i,j] = exp(2G_ij - g_i).

Precision: segment sums run on fp8 e4m3 pixels (segment means average
~256 pixels, so the 3% per-element quantization washes out to ~0.2%);
the conv and gather run bf16 with fp32 PSUM accumulation; everything
downstream of the segment sums (means, adjacency, table build) is fp32.
Output is bf16 (adds <2^-9 relative, vs the 2e-2 gate).
"""

import os
import sys

import numpy as np


def _ensure_path():
    try:
        import concourse  # noqa: F401
    except ImportError:
        for p in ("/opt/trn_rl_repo", os.path.expanduser("~/.axon_site/_ro/trn_rl_repo")):
            if os.path.isdir(p) and p not in sys.path:
                sys.path.insert(0, p)


_ensure_path()
# persistent jax/XLA executable cache: makes repeat compiles of the same
# kernel cheap across processes (first compile of a variant is ~minutes).
os.environ.setdefault("JAX_COMPILATION_CACHE_DIR", "/tmp/jax_neff_cache")
os.environ.setdefault("JAX_PERSISTENT_CACHE_MIN_COMPILE_TIME_SECS", "10")

import concourse.bass as bass  # noqa: E402
import concourse.tile as tile  # noqa: E402
from concourse import bacc  # noqa: E402
from concourse import mybir  # noqa: E402
from concourse.masks import make_identity  # noqa: E402

F32 = mybir.dt.float32

# --- workaround: this walrus build rejects instructions carrying >2 sem
# waits ("Too many sync wait commands" in setupSyncWait). TileContext's exit
# drain accumulates one wait per outstanding processor (DMA queues etc.), so
# split them across NOPs emitted just before the drain. Semaphores are
# monotonic, so waiting earlier on the same conditions is equivalent.
_MAX_WAITS = 1
_drain_patched = False


def _patch_tile_drain():
    global _drain_patched
    if _drain_patched:
        return
    _drain_patched = True
    from concourse.vector_clock import ScopedClock

    orig = tile.TileContext._drain_and_barrier

    def patched(self, tick_clock, wait_clock):
        nc = self.nc
        probe = nc.sync.nop()
        wait_clock.add_sem_waits(
            probe.ins, ScopedClock({None: tick_clock.global_clock})
        )
        waits = list(probe.ins.sync_info.on_wait or [])
        chunks = [waits[i:i + _MAX_WAITS] for i in range(0, len(waits), _MAX_WAITS)]
        probe.ins.sync_info.on_wait = chunks[0] if chunks else []
        for chunk in chunks[1:]:
            nop = nc.sync.nop()
            nop.ins.sync_info = mybir.SyncInfo(on_wait=chunk, on_update=[])
        orig(self, tick_clock, wait_clock)
        _trim_redundant_waits(nc)

    tile.TileContext._drain_and_barrier = patched


def _trim_redundant_waits(nc):
    """Transitive wait reduction. Tile's add_semaphores is per-instruction
    minimal but not transitively minimal across processors: an instruction
    often carries waits already implied by (a) an earlier wait on the same
    engine, or (b) the closure of another wait it carries (the producer's own
    waits + in-order retirement on the producer's engine). This walrus build
    rejects instructions with >2 sync waits, so prune implied waits.

    Soundness assumptions: sem updates fire at instruction retirement;
    retirement is in-order per compute engine and per DMA queue sem (one sem
    per queue, FIFO); a kept wait on sem S>=v implies the v-reaching update's
    instruction retired, hence its dispatch-time holds and (non-DMA) all
    earlier same-engine updates.
    """
    import bisect

    for blk in nc.m.functions[0].blocks:
        insts = list(blk.instructions)
        n = len(insts)
        # sems that are ever decremented/reset are not monotonic; leave all
        # waits on them untouched and exclude them from closures (barrier
        # gather/release sems, end-of-kernel sem clears).
        nonmono = set()
        for ins in insts:
            si = ins.sync_info
            if si and si.on_update:
                for u in si.on_update:
                    if u.update_mode != "sem-inc":
                        nonmono.add(u.id)
            try:
                if ins.is_reset_sema:
                    lo = ins.reset_range_start
                    hi = ins.reset_range_stop
                    if lo is not None and hi is not None:
                        nonmono.update(range(lo, hi + 1))
            except Exception:
                pass
        upd = {}
        cum = {}
        own_cum_after = [None] * n
        eng_of = [str(i.engine) for i in insts]
        is_dma = [type(i).__name__ == "InstDMACopy" for i in insts]
        for idx, ins in enumerate(insts):
            si = ins.sync_info
            d = {}
            if si and si.on_update:
                for u in si.on_update:
                    if (u.update_mode != "sem-inc" or not u.update_value
                            or u.id in nonmono):
                        continue
                    c = cum.get(u.id, 0) + u.update_value
                    cum[u.id] = c
                    upd.setdefault(u.id, []).append((c, idx))
                    d[u.id] = c
            own_cum_after[idx] = d
        eng_cum_after = [None] * n
        run = {}
        for idx in range(n):
            e = eng_of[idx]
            m = dict(run.get(e, {}))
            if not is_dma[idx]:
                for s, c in own_cum_after[idx].items():
                    m[s] = c
            run[e] = m
            eng_cum_after[idx] = m

        def updater_idx(sem, v):
            lst = upd.get(sem)
            if not lst:
                return None
            pos = bisect.bisect_left(lst, (v, -1))
            if pos == len(lst):
                return None
            return lst[pos][1]

        holds_at = [None] * n
        last_eng = {}
        memo = {}

        def completion_holds(uidx):
            if uidx in memo:
                return memo[uidx]
            h = dict(holds_at[uidx] or {})
            src_cum = own_cum_after[uidx] if is_dma[uidx] else eng_cum_after[uidx]
            for s, c in src_cum.items():
                if h.get(s, 0) < c:
                    h[s] = c
            memo[uidx] = h
            return h

        n_dropped = 0
        for idx, ins in enumerate(insts):
            e = eng_of[idx]
            base = dict(holds_at[last_eng[e]]) if e in last_eng else {}
            si = ins.sync_info
            if si and si.on_wait:
                kept = []
                for w in si.on_wait:
                    if w.wait_mode != "sem-ge-imm" or w.id in nonmono:
                        kept.append(w)
                        continue
                    if base.get(w.id, 0) >= w.wait_value:
                        n_dropped += 1
                        continue
                    kept.append(w)
                    ui = updater_idx(w.id, w.wait_value)
                    if ui is not None and ui < idx:
                        for s, v in completion_holds(ui).items():
                            if base.get(s, 0) < v:
                                base[s] = v
                    if base.get(w.id, 0) < w.wait_value:
                        base[w.id] = w.wait_value
                if len(kept) != len(si.on_wait):
                    si.on_wait = kept
            holds_at[idx] = base
            last_eng[e] = idx
_compile_patched = False


def _patch_compile_bir():
    """This walrus build accepts at most ONE sync wait per instruction in
    several encodings (S3_LW matmuls, CTRL NoOp/Drain). Tile legitimately
    emits 2 waits on some instructions, so rewrite the serialized BIR just
    before walrus: keep one wait on the instruction and hoist the rest onto
    same-engine NoOps inserted immediately before it (same dispatch point,
    so semantics are unchanged)."""
    global _compile_patched
    if _compile_patched:
        return
    _compile_patched = True
    import orjson

    from concourse import bass2jax, bass_utils

    orig = bass_utils.compile_bir_kernel

    def _split_waits(bir_json: bytes) -> bytes:
        d = orjson.loads(bir_json)
        changed = False
        for fn in d.get("functions", []):
            for blk in fn.get("blocks", []):
                insts = blk.get("instructions", [])
                out = []
                for inst in insts:
                    si = inst.get("sync_info") or {}
                    ow = si.get("on_wait") or []
                    if len(ow) > 1:
                        changed = True
                        for k, w in enumerate(ow[:-1]):
                            out.append({
                                "debug": inst.get("debug", 0),
                                "engine": inst["engine"],
                                "ins": [],
                                "name": f"{inst['name']}-w{k}",
                                "opcode": "NoOp",
                                "outs": [],
                                "sync_info": {"on_update": [],
                                              "on_wait": [w]},
                            })
                        si["on_wait"] = [ow[-1]]
                    out.append(inst)
                blk["instructions"] = out
        return orjson.dumps(d) if changed else bir_json

    def wrapper(bir_json, tmpdir, neff_name="file.neff"):
        return orig(_split_waits(bir_json), tmpdir, neff_name=neff_name)

    bass_utils.compile_bir_kernel = wrapper
    bass2jax.compile_bir_kernel = wrapper


AF = mybir.ActivationFunctionType
ALU = mybir.AluOpType

B, C, K, H, W_DIM = 8, 256, 64, 128, 128
HW = H * W_DIM  # 16384 pixels per sample
N_CORES = 8
NCH = HW // 128       # 128 pixel chunks of 128
P1_G = 8              # pass-1 DMA groups (16 chunks each)
CPG = NCH // P1_G     # chunks per group = 16


def build_nc():
    _patch_tile_drain()
    _patch_compile_bir()
    nc = bacc.Bacc("TRN2", target_bir_lowering=False, debug=False)
    BF16 = mybir.dt.bfloat16
    F8 = mybir.dt.float8e4
    out_d = nc.dram_tensor("out", (128, 2, HW), BF16, kind="ExternalOutput")
    ins = dict(
        x8t=nc.dram_tensor("x8t", (128, NCH * C), F8, kind="ExternalInput").ap(),
        oh1=nc.dram_tensor("oh1", (128, NCH * K), F8, kind="ExternalInput").ap(),
        xh=nc.dram_tensor("xh", (128, 2 * HW), BF16, kind="ExternalInput").ap(),
        idxbf=nc.dram_tensor("idxbf", (HW,), BF16, kind="ExternalInput").ap(),
        recip=nc.dram_tensor("recip", (K, 1), F32, kind="ExternalInput").ap(),
        wt=nc.dram_tensor("wt", (C, C), F32, kind="ExternalInput").ap(),
        cwth=nc.dram_tensor("cwth", (C, C), BF16, kind="ExternalInput").ap(),
    )

    with tile.TileContext(nc) as tc:
        _body(tc, ins, out_d.ap())
    nc.compile()
    return nc


def _body(tc, ins, out_v):
    nc = tc.nc
    BF16 = mybir.dt.bfloat16
    F8 = mybir.dt.float8e4

    with (
        tc.tile_pool(name="consts", bufs=1) as consts,
        tc.tile_pool(name="xres", bufs=P1_G) as xres,
        tc.tile_pool(name="mid_sb", bufs=1) as mid_sb,
    ):
        # ---- constants / parameter loads ----
        ident = consts.tile([128, 128], F32, tag="ident")
        make_identity(nc, ident[:])

        wt_sb = consts.tile([128, 2, C], F32, tag="wt_sb")     # [e, j, c] = W^T[j*128+e, c]
        nc.sync.dma_start(out=wt_sb[:], in_=ins["wt"].rearrange("(j p) c -> p j c", p=128))
        cwth_sb = consts.tile([128, 2, C], BF16, tag="cwth_sb")
        nc.sync.dma_start(
            out=cwth_sb[:],
            in_=ins["cwth"].rearrange("(j p) c -> p j c", p=128))
        recip_sb = consts.tile([K, 1], F32, tag="recip_sb")
        nc.sync.dma_start(out=recip_sb[:], in_=ins["recip"])

        iota_col = consts.tile([K, 1], F32, tag="iota_col")    # [k,0] = k
        iota_col_i = consts.tile([K, 1], mybir.dt.int32, tag="iota_col_i")
        nc.gpsimd.iota(iota_col_i[:], pattern=[[1, 1]], base=0,
                       channel_multiplier=1)
        nc.vector.tensor_copy(iota_col[:], iota_col_i[:])
        # index values broadcast to K partitions, and the pass-2 one-hot
        # [k, px] built from it during pass-1's vector slack.
        idx_bc = consts.tile([K, HW], BF16, tag="idx_bc")
        oh2_all = consts.tile([K, HW], BF16, tag="oh2_all")

        M_sb = mid_sb.tile([128, 2, C], F32, tag="M_sb")       # M = W @ W^T (symmetric)
        means = mid_sb.tile([K, C], F32, tag="means")
        meansT = mid_sb.tile([128, 2, K], F32, tag="meansT")
        meansT_h = mid_sb.tile([128, 2, K], BF16, tag="meansT_h")
        Q_sb = mid_sb.tile([128, 2, K], F32, tag="Q_sb")
        aggT_h = mid_sb.tile([128, 2, K], BF16, tag="aggT_h")
        B_sb = mid_sb.tile([K, K], F32, tag="B_sb")
        tmp64 = mid_sb.tile([K, K], F32, tag="tmp64")
        negI = consts.tile([K, K], F32, tag="negI")            # -identity(64)
        nc.gpsimd.memset(negI[:], 0.0)
        nc.gpsimd.affine_select(
            out=negI[:], in_=negI[:], compare_op=ALU.not_equal,
            fill=-1.0, base=0, pattern=[[-1, K]], channel_multiplier=1,
        )
        scratch64 = mid_sb.tile([K, 1], F32, tag="scratch64")
        neg_g = mid_sb.tile([K, 1], F32, tag="neg_g")
        e_col = mid_sb.tile([K, 1], F32, tag="e_col")
        tableM = mid_sb.tile([K, C], F32, tag="tableM")
        table2T = mid_sb.tile([K, C], F32, tag="table2T")
        tabh = mid_sb.tile([K, C], BF16, tag="tabh")

        x_tiles = []

        with (
            tc.tile_pool(name="p1_sb", bufs=1) as p1_sb,
            tc.tile_pool(name="psum_sums", bufs=1, space="PSUM") as pp_sums,
            tc.tile_pool(name="psum_mid", bufs=2, space="PSUM") as pp_mid,
            tc.tile_pool(name="psum_fill", bufs=1, space="PSUM") as pp_fill,
        ):
            # pixel-major fp8 x (with c contiguous per chunk) and one-hot.
            # Transfers are partition-split so each logical load spreads over
            # several DMA queues (per-queue streaming is the bottleneck).
            x8t_sb = p1_sb.tile([128, P1_G, CPG, C], F8, tag="x8t_sb")
            oh1_sb = p1_sb.tile([128, P1_G, CPG, K], F8, tag="oh1_sb")
            x8_r = ins["x8t"].rearrange("p (g a c) -> p g a c", g=P1_G, a=CPG)
            oh_r = ins["oh1"].rearrange("p (g a k) -> p g a k", g=P1_G, a=CPG)
            # pass-1 feeds go first so they win the DMA queues
            for q in range(4):
                ps = slice(q * 32, (q + 1) * 32)
                nc.sync.dma_start(out=x8t_sb[ps, 0, :, :], in_=x8_r[ps, 0, :, :])
            for q in range(4):
                ps = slice(q * 32, (q + 1) * 32)
                nc.sync.dma_start(out=oh1_sb[ps, :, :, :], in_=oh_r[ps, :, :, :])
            for g in range(1, P1_G):
                for q in range(4):
                    ps = slice(q * 32, (q + 1) * 32)
                    nc.sync.dma_start(
                        out=x8t_sb[ps, g, :, :], in_=x8_r[ps, g, :, :])
            nc.sync.dma_start(
                out=idx_bc[:],
                in_=ins["idxbf"].unsqueeze(0).to_broadcast((K, HW)))

            psum_sums = pp_sums.tile([K, C], F32, tag="psum_sums")

            # HAM keep-alive: the PE clock gate (PE_HAM) demotes to K=4/8
            # when non-transpose matmul activity dips, and once pass 2 runs
            # at the gated clock it never re-promotes. These dep-free filler
            # matmuls into a scratch bank keep measured activity high across
            # the (otherwise PE-idle) middle stage.
            fscr = pp_fill.tile([128, 512], F32, tag="fscr")
            fill_src = []

            def filler(n):
                if not fill_src:
                    return
                xt0 = fill_src[0]
                for _ in range(n):
                    nc.tensor.matmul(
                        fscr[:], cwth_sb[:, 0, 0:128], xt0[:, 0, 0:512],
                        start=True, stop=True)

            # Warm-up: make PE observe the POOL-produced identity before the
            # hot loop.
            warm = pp_mid.tile([128, C], F32, tag="pm")
            nc.tensor.transpose(warm[:, 0:128], ident[:], ident[:])

            # M = W @ W^T: contract e; lhsT/rhs both W^T (e on partitions).
            for h in range(2):
                pm = pp_mid.tile([128, C], F32, tag="pm")
                for j in range(2):
                    nc.tensor.matmul(
                        pm[:], wt_sb[:, j, h * 128:(h + 1) * 128],
                        wt_sb[:, j, :], start=(j == 0), stop=(j == 1),
                    )
                nc.scalar.copy(M_sb[:, h, :], pm[:])

            # preload the Exp activation table so the middle stage does not
            # pay the ~1.3us table load on its critical path
            nc.scalar.activation(scratch64[:], neg_g[:], AF.Exp)

            # ---- pass 1: segment sums over all pixels ----
            xh_r = ins["xh"].rearrange("p (g j w) -> p g j w", g=P1_G, j=2)
            for g in range(P1_G):
                # resident x for pass 2: queued per-group so these DMAs stay
                # behind the pass-1 feeds of later groups. Host layout keeps
                # each partition's tile contiguous (8 KB descriptors).
                xt_h = xres.tile([128, 2, HW // P1_G], BF16, tag="xres")
                x_tiles.append(xt_h)
                if not fill_src:
                    fill_src.append(xt_h)
                for q in range(2):
                    ps = slice(q * 64, (q + 1) * 64)
                    nc.sync.dma_start(
                        out=xt_h[ps, :, :], in_=xh_r[ps, g, :, :])
                # pass-2 one-hot slice for this group's pixels on the vector
                # engine (idle capacity during pass 1)
                sl = slice(g * (HW // P1_G), (g + 1) * (HW // P1_G))
                nc.vector.tensor_scalar(
                    out=oh2_all[:, sl], in0=idx_bc[:, sl],
                    scalar1=iota_col[:], scalar2=None, op0=ALU.is_equal)
                for a in range(CPG):
                    ch = g * CPG + a
                    nc.tensor.matmul(
                        psum_sums[:], oh1_sb[:, g, a, :], x8t_sb[:, g, a, :],
                        start=(ch == 0), stop=(ch == NCH - 1))

            # ---- middle: means -> adjacency -> table ----
            nc.vector.tensor_scalar(
                out=means[:], in0=psum_sums[:], scalar1=recip_sb[:],
                scalar2=None, op0=ALU.mult,
            )
            filler(2)

            # meansT (c on partitions)
            for h in range(2):
                pm = pp_mid.tile([128, C], F32, tag="pm")
                nc.tensor.transpose(
                    pm[:, 0:K], means[:, h * 128:(h + 1) * 128], ident[0:K, 0:K],
                )
                nc.scalar.copy(meansT[:, h, :], pm[:, 0:K])
                filler(2)
            nc.vector.tensor_copy(meansT_h[:], meansT[:])

            # Q = M @ means^T  (use symmetry of M for lhsT slicing)
            for h in range(2):
                pq = pp_mid.tile([128, C], F32, tag="pm")
                for dj in range(2):
                    nc.tensor.matmul(
                        pq[:, 0:K], M_sb[:, dj, h * 128:(h + 1) * 128],
                        meansT[:, dj, :], start=(dj == 0), stop=(dj == 1),
                    )
                nc.scalar.copy(Q_sb[:, h, :], pq[:, 0:K])
                filler(2)

            # G = means @ Q  (64x64, symmetric)
            pg = pp_mid.tile([128, C], F32, tag="pm")
            for h in range(2):
                nc.tensor.matmul(
                    pg[0:K, 0:K], meansT[:, h, :], Q_sb[:, h, :],
                    start=(h == 0), stop=(h == 1),
                )
            filler(4)

            # -g = rowsum(G * (-I));  e_col = exp(-g);  B = exp(2G - g_i)
            nc.vector.scalar_tensor_tensor(
                out=tmp64[:], in0=pg[0:K, 0:K], scalar=1.0, in1=negI[:],
                op0=ALU.mult, op1=ALU.mult, accum_out=neg_g[:],
            )
            nc.scalar.activation(e_col[:], neg_g[:], AF.Exp)
            nc.scalar.activation(B_sb[:], pg[0:K, 0:K], AF.Exp, bias=neg_g[:], scale=2.0)
            filler(4)

            # aggT_raw[c,i] = sum_j B[j,i] means[j,c]
            # (B[j,i] = exp(2G_ij - g_j) already carries e^{-g_j})
            for h in range(2):
                pa = pp_mid.tile([128, C], F32, tag="pm")
                nc.tensor.matmul(
                    pa[:, 0:K], means[:, h * 128:(h + 1) * 128], B_sb[:],
                    start=True, stop=True,
                )
                nc.vector.tensor_copy(aggT_h[:, h, :], pa[:, 0:K])
                filler(2)
            # table2T[k, c_out] = e^{-g_k}*(aggT_raw^T@cwt)[k,:] - means@cwt
            # (bf16 weights: same quantized conv_w as pass 2 uses)
            pt2 = pp_mid.tile([128, C], F32, tag="pm")
            ptm = pp_mid.tile([128, C], F32, tag="pm")
            for j in range(2):
                nc.tensor.matmul(
                    pt2[0:K, :], aggT_h[:, j, :], cwth_sb[:, j, :],
                    start=(j == 0), stop=(j == 1),
                )
            for j in range(2):
                nc.tensor.matmul(
                    ptm[0:K, :], meansT_h[:, j, :], cwth_sb[:, j, :],
                    start=(j == 0), stop=(j == 1),
                )
            nc.scalar.copy(tableM[:], ptm[0:K, :])
            filler(4)
            nc.vector.scalar_tensor_tensor(
                out=table2T[:], in0=pt2[0:K, :], scalar=e_col[:], in1=tableM[:],
                op0=ALU.mult, op1=ALU.subtract,
            )
            # bf16 table for the gather matmul
            nc.vector.tensor_copy(tabh[:], table2T[:])
            filler(5)

        # ---- pass 2: out = conv_w @ x + table[index] ----
        # Weight-grouped: per 1024-px group each lhsT serves two back-to-back
        # 512-col matmuls, halving LDWEIGHTS pressure vs per-512 tiles.
        # Output staged per 2048 px so DMA descriptors are 4 KB.
        PXG = 1024
        n_g = HW // PXG                        # 16
        tile_px = HW // P1_G                   # 2048
        with (
            tc.tile_pool(name="psum_p2", bufs=2, space="PSUM") as pp2,
            tc.tile_pool(name="p2_sb", bufs=3) as p2_sb,
        ):
            for gg in range(n_g // 2):
                ot = p2_sb.tile([128, 2, tile_px], BF16, tag="ot")
                for sub in range(2):
                    g2 = gg * 2 + sub
                    ti = (g2 * PXG) // tile_px
                    off = (g2 * PXG) % tile_px
                    xt_h = x_tiles[ti]
                    for h in range(2):
                        hs = slice(h * 128, (h + 1) * 128)
                        po = pp2.tile([128, PXG], F32, tag=f"po{h}")
                        for j in range(2):
                            for cc in range(2):
                                nc.tensor.matmul(
                                    po[:, cc * 512:(cc + 1) * 512],
                                    cwth_sb[:, j, hs],
                                    xt_h[:, j,
                                         off + cc * 512:off + (cc + 1) * 512],
                                    start=(j == 0), stop=False)
                        for cc in range(2):
                            nc.tensor.matmul(
                                po[:, cc * 512:(cc + 1) * 512], tabh[:, hs],
                                oh2_all[:, g2 * PXG + cc * 512:
                                        g2 * PXG + (cc + 1) * 512],
                                start=False, stop=True)
                        osl = slice(sub * PXG, (sub + 1) * PXG)
                        if h == 0:
                            nc.scalar.copy(ot[:, h, osl], po[:])
                        else:
                            nc.vector.tensor_copy(ot[:, h, osl], po[:])
                for h in range(2):
                    nc.sync.dma_start(
                        out=out_v[:, h, gg * tile_px:(gg + 1) * tile_px],
                        in_=ot[:, h, :])


def _ensure_ntff_hook():
    """Register the axon NTFF profiling hook if the image's antenv lacks it."""
    try:
        from antenv.axon_hooks import get_axon_ntff_profile_hook  # noqa: F401
        return
    except ImportError:
        pass
    import types

    import antenv

    mod = types.ModuleType("antenv.axon_hooks")
    _hook = [None]
    mod.set_axon_ntff_profile_hook = lambda h: _hook.__setitem__(0, h)
    mod.get_axon_ntff_profile_hook = lambda: _hook[0]
    sys.modules["antenv.axon_hooks"] = mod
    antenv.axon_hooks = mod
    try:
        from trn_agent_boot.trn_boot import _ntff_profile_via_ctypes

        so = "/opt/axon/libaxon_pjrt.so"
        if os.path.exists(so):
            mod.set_axon_ntff_profile_hook(_ntff_profile_via_ctypes(so))
    except Exception:
        pass


_NC_CACHE = None
LAST_RESULT = None


def _get_nc():
    global _NC_CACHE
    if _NC_CACHE is None:
        _NC_CACHE = build_nc()
    return _NC_CACHE


def kernel(x, index, W, conv_w):
    """Full inputs in, full output out. Shards batch across 8 NeuronCores."""
    global LAST_RESULT
    from concourse.bass_utils import run_bass_kernel_spmd

    import ml_dtypes

    F8NP = ml_dtypes.float8_e4m3
    x = np.asarray(x, dtype=np.float32).reshape(B, C, HW)
    idx_i = np.asarray(index).reshape(B, HW)
    idxf = idx_i.astype(np.float32)
    wt = np.ascontiguousarray(np.asarray(W, dtype=np.float32).T)
    cwt = np.ascontiguousarray(
        np.asarray(conv_w, dtype=np.float32).reshape(C, C).T
    )

    nc = _get_nc()
    # xh[b, p, g*2*TP + j*TP + w] = x[b, j*128+p, g*TP + w]: per-partition
    # contiguous per (group, j) so DMA descriptors are 8 KB
    TP = HW // P1_G
    xh = np.ascontiguousarray(
        x.reshape(B, 2, 128, P1_G, TP).transpose(0, 2, 3, 1, 4)
    ).astype(ml_dtypes.bfloat16).reshape(B, 128, 2 * HW)
    cwth = cwt.astype(ml_dtypes.bfloat16)
    idxbf = idxf.astype(ml_dtypes.bfloat16)
    # pixel-major layouts for pass 1, partition-contiguous in DRAM:
    #   x8t[b, p, a*C + c] = x[b, c, a*128 + p]   (fp8 e4m3, clipped)
    #   oh1[b, p, a*K + k] = (index[b, a*128+p] == k)
    x8t_all = np.clip(x.reshape(B, C, NCH, 128).transpose(0, 3, 2, 1),
                      -240.0, 240.0).astype(F8NP).reshape(B, 128, NCH * C)
    idx_pm = idx_i.reshape(B, NCH, 128).transpose(0, 2, 1)  # [b, p, a]
    oh1_all = (idx_pm[..., None] == np.arange(K)[None, None, None, :]).astype(
        F8NP).reshape(B, 128, NCH * K)
    counts = np.stack([np.bincount(idx_i[b], minlength=K) for b in range(B)])
    recip = (1.0 / np.maximum(counts, 1)).astype(np.float32)[..., None]

    in_maps = [
        {"x8t": np.ascontiguousarray(x8t_all[b]),
         "oh1": np.ascontiguousarray(oh1_all[b]),
         "xh": np.ascontiguousarray(xh[b]),
         "idxbf": np.ascontiguousarray(idxbf[b]),
         "recip": np.ascontiguousarray(recip[b]),
         "wt": wt, "cwth": cwth}
        for b in range(B)
    ]
    trace = bool(int(os.environ.get("KERNEL_TRACE", "0")))
    if trace:
        try:
            _ensure_ntff_hook()
            res = run_bass_kernel_spmd(
                nc, in_maps, core_ids=list(range(N_CORES)), trace=True,
            )
        except Exception as e:  # profiling must never break the answer path
            print(f"kernel: trace run failed ({e!r}); rerunning untraced")
            res = run_bass_kernel_spmd(
                nc, in_maps, core_ids=list(range(N_CORES)), trace=False,
            )
    else:
        res = run_bass_kernel_spmd(
            nc, in_maps, core_ids=list(range(N_CORES)), trace=False,
        )
    LAST_RESULT = res
    out = np.stack([
        np.asarray(res.results[b]["out"]).astype(np.float32)
        .reshape(128, 2, HW).transpose(1, 0, 2).reshape(C, H, W_DIM)
        for b in range(B)
    ])
    return out
